# revision 8
# baseline (speedup 1.0000x reference)
"""GwcVolume (group-wise correlation cost volume) Trainium2 kernel.

cost[b,g,d,h,x] = mean_c( lf[b, g*8+c, h, x] * rf[b, g*8+c, h, x-d] ),
zero for x < d.  Shapes: lf/rf [2, 320, 128, 240] f32 -> out [2, 40, 48, 128, 240] f32.

Sharding: h-axis split across 8 cores (16 rows each). Correlation is along w
only, so shards are fully independent and each core reads just its h-band.

Per-core engine mapping (v2 — derived from the v1 trace, which showed every
engine near-saturated around the DVE product floor):
  - SWDGE DMA loads lf/rf h-band once, casting fp32->fp16 in flight, with NO
    left-pad (big contiguous descriptors). The 64-channel tail chunk is
    h-paired into [128, 4, W] so all 128 DVE lanes / PE rows stay busy.
  - The x<d output region is skipped at even-d granularity: products,
    matmuls and drains cover x in [d&~1, W) only. With that, the rf window
    always starts at column 0: even d reads rf directly; odd d reads a
    one-column-right-shifted copy (made on ScalarE) whose column 0 is zero.
    All DVE operands stay 4B-aligned so the 2x perf mode always engages.
  - VectorE computes lf*rf products in fp16 (2x mode).
  - TensorE reduces the 8 channels of each group via constant 0/1*(1/8)
    block matrices col-packed at tile_position (0,0)/(0,32)/(0,64); PSUM
    output lands on partitions 0:16/32:48/64:80.
  - ScalarE drains PSUM->SBUF into a 4-disparity staging tile, restoring the
    x<d zeros from a zero tile; HWDGE writes ~0.5MB per output DMA (4 d's at
    once) to cut per-trigger sync-engine cost.
"""

import numpy as np

import concourse.bass as bass
import concourse.tile as tile
from concourse import mybir
from concourse.bass_utils import run_bass_kernel_spmd

B = 2
C = 320
H = 128
W = 240
G = 40
CPG = 8
D = 48
NCORES = 8
HS = H // NCORES  # h rows per core
HB = 8  # h rows per inner block
ND = 4  # disparities per output staging block
F16 = mybir.dt.float16
F32 = mybir.dt.float32


def split_multi_waits(nc, limit=1):
    """Walrus in this container rejects instructions carrying more than
    `limit` semaphore waits. Move excess waits onto preceding NoOps on the
    same engine (waits execute before the instruction, in stream order)."""
    n_split = 0
    for fn in nc.m.functions:
        for bb in fn.blocks:
            insts = bb.instructions
            i = 0
            while i < len(insts):
                inst = insts[i]
                si = inst.sync_info
                if si is not None and len(si.on_wait) > limit:
                    waits = list(si.on_wait)
                    keep = waits[-limit:]
                    extra = waits[:-limit]
                    new_insts = []
                    for j in range(0, len(extra), limit):
                        chunk = extra[j : j + limit]
                        nop = mybir.InstNoOp(
                            name=nc.get_next_instruction_name(),
                            engine=inst.engine,
                            ins=[],
                            outs=[],
                            sync_info=mybir.SyncInfo(on_wait=chunk, on_update=[]),
                        )
                        new_insts.append(nop)
                    inst.sync_info = mybir.SyncInfo(
                        on_wait=keep, on_update=list(si.on_update)
                    )
                    insts[i:i] = new_insts
                    i += len(new_insts)
                    n_split += 1
                i += 1
    return n_split


def build_bass(n_b=B, n_hb=HS // HB, n_d=D):
    nc = bass.Bass("TRN2", target_bir_lowering=False, debug=False, num_devices=NCORES)
    lf = nc.dram_tensor("lf", [B, C, HS, W], F32, kind="ExternalInput").ap()
    rf = nc.dram_tensor("rf", [B, C, HS, W], F32, kind="ExternalInput").ap()
    s16 = nc.dram_tensor("s16", [128, 16], F16, kind="ExternalInput").ap()
    s8p = nc.dram_tensor("s8p", [128, 16], F16, kind="ExternalInput").ap()
    outp = nc.dram_tensor("outp", [B, G, D, HS, W], F32, kind="ExternalOutput").ap()

    n_dblk = (n_d + ND - 1) // ND

    with tile.TileContext(nc) as tc:
        with (
            tc.tile_pool(name="const", bufs=1) as cpool,
            tc.tile_pool(name="lf", bufs=2) as lpool,
            tc.tile_pool(name="rf", bufs=2) as rpool,
            tc.tile_pool(name="prod", bufs=3) as ppool,
            tc.tile_pool(name="outs", bufs=2) as opool,
            tc.tile_pool(name="psum", bufs=2, space="PSUM") as qpool,
        ):
            s16_t = cpool.tile([128, 16], F16)
            nc.sync.dma_start(s16_t[:], s16[:])
            s8p_t = cpool.tile([128, 16], F16)
            nc.sync.dma_start(s8p_t[:], s8p[:])
            s_tiles = [s16_t, s16_t, s8p_t]

            for b in range(n_b):
                for hg in range(n_hb):
                    h0 = hg * HB
                    # chunks 1-2: [128ch, 8h, W]; chunk3 h-paired: [2hp*64ch, 4h, W]
                    lf_ts, rf_e_ts, rf_o_ts = [], [], []
                    for c0 in (0, 128):
                        lt = lpool.tile([128, HB, W], F16, tag=f"lf{c0}")
                        nc.gpsimd.dma_start(lt[:, :, :], lf[b, c0 : c0 + 128, h0 : h0 + HB, :])
                        re = rpool.tile([128, HB, W], F16, tag=f"rfe{c0}")
                        nc.gpsimd.dma_start(re[:, :, :], rf[b, c0 : c0 + 128, h0 : h0 + HB, :])
                        ro = rpool.tile([128, HB, 242], F16, tag=f"rfo{c0}")
                        nc.gpsimd.memset(ro[:, :, 0:1], 0.0)
                        nc.scalar.copy(ro[:, :, 1:241], re[:, :, :])
                        lf_ts.append(lt)
                        rf_e_ts.append(re)
                        rf_o_ts.append(ro)
                    hh = HB // 2
                    lt = lpool.tile([128, hh, W], F16, tag="lf256")
                    nc.gpsimd.dma_start(lt[0:64, :, :], lf[b, 256:320, h0 : h0 + hh, :])
                    nc.gpsimd.dma_start(lt[64:128, :, :], lf[b, 256:320, h0 + hh : h0 + HB, :])
                    re = rpool.tile([128, hh, W], F16, tag="rfe256")
                    nc.gpsimd.dma_start(re[0:64, :, :], rf[b, 256:320, h0 : h0 + hh, :])
                    nc.gpsimd.dma_start(re[64:128, :, :], rf[b, 256:320, h0 + hh : h0 + HB, :])
                    ro = rpool.tile([128, hh, 242], F16, tag="rfo256")
                    nc.gpsimd.memset(ro[:, :, 0:1], 0.0)
                    nc.scalar.copy(ro[:, :, 1:241], re[:, :, :])
                    lf_ts.append(lt)
                    rf_e_ts.append(re)
                    rf_o_ts.append(ro)

                    # d runs high->low: each og slot sees d decreasing by
                    # 2*ND on reuse, so its x<d zero region only shrinks and
                    # one memset per block keeps it zero (no per-d zero fill)
                    for blk_i, dblk in enumerate(reversed(range(n_dblk))):
                        nd_i = min(ND, n_d - dblk * ND)
                        # f16 staging: ScalarE casts on PSUM drain, DGE casts
                        # back to f32 on the DRAM write (halves SBUF-side
                        # DMA traffic of the 59MB output)
                        og = opool.tile([80, ND, HB, W], F16)
                        if blk_i < 2:
                            nc.gpsimd.memset(og[:, :, :, :], 0.0)
                        for i in range(nd_i):
                            d = dblk * ND + i
                            d0 = d & ~1
                            wv = W - d0
                            prods = []
                            for ci in range(3):
                                rows = HB if ci < 2 else hh
                                pt = ppool.tile([128, rows, W], F16, tag=f"prod{ci}")
                                src = rf_e_ts[ci] if d % 2 == 0 else rf_o_ts[ci]
                                nc.vector.tensor_mul(
                                    pt[:, :, d0:W],
                                    lf_ts[ci][:, :, d0:W],
                                    src[:, :, 0:wv],
                                )
                                prods.append(pt)
                            ps = qpool.tile([80, HB, 256], F32)
                            for ci, strip in enumerate((0, 32, 64)):
                                rows = HB if ci < 2 else hh
                                for j in range(rows // 2):
                                    nc.tensor.matmul(
                                        ps[strip : strip + 16, 2 * j : 2 * j + 2, d0:W],
                                        s_tiles[ci][:, :],
                                        prods[ci][:, 2 * j : 2 * j + 2, d0:W],
                                        start=True,
                                        stop=True,
                                        tile_position=(0, strip),
                                    )
                            nc.scalar.copy(og[:, i, :, d0:W], ps[:, :, d0:W])
                        # SWDGE (gpsimd) triggers: descriptor generation runs
                        # on the idle Q7 instead of serializing the sync
                        # engine at ~63ns/descriptor
                        dlo = dblk * ND
                        nc.gpsimd.dma_start(
                            outp[b, 0:16, dlo : dlo + nd_i, h0 : h0 + HB, :],
                            og[0:16, 0:nd_i, :, :],
                        )
                        nc.gpsimd.dma_start(
                            outp[b, 16:32, dlo : dlo + nd_i, h0 : h0 + HB, :],
                            og[32:48, 0:nd_i, :, :],
                        )
                        nc.gpsimd.dma_start(
                            outp[b, 32:40, dlo : dlo + nd_i, h0 : h0 + hh, :],
                            og[64:72, 0:nd_i, 0:hh, :],
                        )
                        nc.gpsimd.dma_start(
                            outp[b, 32:40, dlo : dlo + nd_i, h0 + hh : h0 + HB, :],
                            og[72:80, 0:nd_i, 0:hh, :],
                        )
    split_multi_waits(nc)
    return nc


def make_smats():
    s16 = np.zeros((128, 16), np.float16)
    for g in range(16):
        s16[g * CPG : (g + 1) * CPG, g] = 1.0 / CPG
    # h-paired tail chunk: partition hp*64+c'' (channel 256+c'', h-half hp)
    # -> output column hp*8+g
    s8p = np.zeros((128, 16), np.float16)
    for hp in range(2):
        for g in range(8):
            s8p[hp * 64 + g * CPG : hp * 64 + (g + 1) * CPG, hp * 8 + g] = 1.0 / CPG
    return s16, s8p


_NC_CACHE = {}


def _get_nc(key=(B, HS // HB, D)):
    if key not in _NC_CACHE:
        _NC_CACHE[key] = build_bass(*key)
    return _NC_CACHE[key]


def run_sharded(lf, rf, nc=None, trace=False):
    """lf/rf: full [2, 320, 128, 240] f32 numpy arrays. Returns (out, results)."""
    if nc is None:
        nc = _get_nc()
    s16, s8p = make_smats()
    in_maps = []
    for k in range(NCORES):
        in_maps.append(
            {
                "lf": np.ascontiguousarray(lf[:, :, k * HS : (k + 1) * HS, :]),
                "rf": np.ascontiguousarray(rf[:, :, k * HS : (k + 1) * HS, :]),
                "s16": s16,
                "s8p": s8p,
            }
        )
    res = run_bass_kernel_spmd(nc, in_maps, list(range(NCORES)), trace=trace)
    out = np.empty((B, G, D, H, W), np.float32)
    for k in range(NCORES):
        out[:, :, :, k * HS : (k + 1) * HS, :] = res.results[k]["outp"]
    return out, res


def kernel(**inputs):
    lf = np.asarray(inputs["left_feature"], dtype=np.float32)
    rf = np.asarray(inputs["right_feature"], dtype=np.float32)
    out, _ = run_sharded(lf, rf)
    return out


if __name__ == "__main__":
    rng = np.random.default_rng(0)
    lf = rng.standard_normal((B, C, H, W), dtype=np.float32)
    rf = rng.standard_normal((B, C, H, W), dtype=np.float32)
    out, _ = run_sharded(lf, rf)
    print(out.shape, out.dtype, float(np.abs(out).max()))
